# revision 34
# baseline (speedup 1.0000x reference)
"""Self-contained TRN2 Bass kernel for causal multi-head attention.

Problem: x[4,2048,1024], causal mask, wq/wk/wv/wo [1024,1024], H=16, HD=64.
Sharding: 8 NeuronCores = 4 batches x 2 head-groups (8 heads each).
Each core computes Q/K/V projections for its head group (bf16), causal
attention with block-skipping + diagonal trimming, and a partial o_proj;
the two partial outputs per batch are summed on host (the tensor-parallel
all-reduce of the unshard step).

v2: bf16 datapath, single x pass, proj/attention/o_proj software-pipelined
per 512-query block, triangular-diag masking at 128x128, no gpsimd on the
critical path except the denominator broadcast.
"""
import sys
sys.path.insert(0, "/opt/trn_rl_repo")

import numpy as np

from contextlib import ExitStack

import concourse.bass as bass
import concourse.mybir as mybir
import concourse.tile as tile
from concourse import bacc

f32 = mybir.dt.float32
bf16 = mybir.dt.bfloat16
EXP = mybir.ActivationFunctionType.Exp


def build(T=2048, C=1024, HL=8, D=64, types=None, mixidx=None, nmix=0,
          n_cores=8, debug=False):
    """types[jt][ib]: 0 = skip, 1 = full, 2 = general mixed (mask tile
    mixidx[jt][ib]), 3 = causal diagonal (trim start (jt-4*ib)*128)."""
    OL = HL * D          # 512 out dims per core
    JT = T // 128        # 16 key tiles
    IBN = T // 512       # 4 query blocks
    NC = C // 128        # 8 contraction chunks
    NO = OL // 128       # 4 out chunks
    NP = HL // 2         # 4 head pairs
    SCALE = 1.0 / float(D) ** 0.5
    E = D + 1            # v columns per head incl. trailing ones column

    assert types is not None

    nc = bacc.Bacc("TRN2", target_bir_lowering=False, debug=False,
                   num_devices=n_cores)

    xT = nc.dram_tensor("xT", [C, T], bf16, kind="ExternalInput").ap()
    wqT = nc.dram_tensor("wqT", [C, OL], bf16, kind="ExternalInput").ap()
    wkT = nc.dram_tensor("wkT", [C, OL], bf16, kind="ExternalInput").ap()
    wvT = nc.dram_tensor("wvT", [C, OL], bf16, kind="ExternalInput").ap()
    woT = nc.dram_tensor("woT", [OL, C], bf16, kind="ExternalInput").ap()
    triT = nc.dram_tensor("triT", [128, 128], bf16, kind="ExternalInput").ap()
    maskT = None
    if nmix:
        maskT = nc.dram_tensor("maskT", [nmix, 128, 512], bf16,
                               kind="ExternalInput").ap()
    yT = nc.dram_tensor("yT", [C, T], bf16, kind="ExternalOutput").ap()
    dbg = {}
    if debug:
        NOx = OL // 128
        dbg["qT"] = nc.dram_tensor("dbg_qT", [NOx, 128, T], bf16,
                                   kind="ExternalOutput").ap()
        dbg["kT"] = nc.dram_tensor("dbg_kT", [NOx, 128, T], bf16,
                                   kind="ExternalOutput").ap()
        dbg["v"] = nc.dram_tensor("dbg_v", [JT, 128, HL * E], bf16,
                                  kind="ExternalOutput").ap()
        dbg["aT"] = nc.dram_tensor("dbg_aT", [NOx, 128, T], bf16,
                                   kind="ExternalOutput").ap()

    with tile.TileContext(nc) as tc, ExitStack() as ctx:
        persist = ctx.enter_context(tc.tile_pool(name="persist", bufs=1))
        qT_sb = [persist.tile([128, T], bf16, tag=f"qT{o}", name=f"qT{o}")
                 for o in range(NO)]
        kT_sb = [persist.tile([128, T], bf16, tag=f"kT{o}", name=f"kT{o}")
                 for o in range(NO)]
        v_sb = [persist.tile([128, HL * E], bf16, tag=f"v{t}", name=f"v{t}")
                for t in range(JT)]
        aT_sb = [persist.tile([128, T], bf16, tag=f"aT{o}", name=f"aT{o}")
                 for o in range(NO)]
        # consolidated weight tiles: chunk c lives at cols [c*OL, (c+1)*OL)
        wq_big = persist.tile([128, NC * OL], bf16, tag="wqB", name="wqB")
        wk_big = persist.tile([128, NC * OL], bf16, tag="wkB", name="wkB")
        wv_big = persist.tile([128, NC * OL], bf16, tag="wvB", name="wvB")
        wo_big = persist.tile([128, NO * C], bf16, tag="woB", name="woB")
        wq_sb = [wq_big[:, c * OL:(c + 1) * OL] for c in range(NC)]
        wk_sb = [wk_big[:, c * OL:(c + 1) * OL] for c in range(NC)]
        wv_sb = [wv_big[:, c * OL:(c + 1) * OL] for c in range(NC)]
        wo_sb = [wo_big[:, o * C:(o + 1) * C] for o in range(NO)]
        tri_sb = persist.tile([128, 128], bf16, tag="tri", name="tri")

        # DMA order matters: the first Q-proj matmul group needs only
        # wq_big and xs(tb=0) — front-load those as single big DMAs so the
        # Sync queue issues 2 triggers, not 16.
        px = ctx.enter_context(tc.tile_pool(name="px", bufs=2))
        xs_all = {}
        def load_xs(tb, split=False):
            tis = slice(tb * 512, (tb + 1) * 512)
            xsb = px.tile([128, NC * 512], bf16, tag="xsB", name=f"xsB_{tb}")
            src = xT[:, tis].rearrange("(c p) t -> p c t", p=128)
            dst = xsb[:].rearrange("p (c t) -> p c t", t=512)
            if split:
                nc.sync.dma_start(dst[:, 0:NC // 2], src[:, 0:NC // 2])
                nc.scalar.dma_start(dst[:, NC // 2:NC], src[:, NC // 2:NC])
            else:
                nc.sync.dma_start(dst, src)
            xs_all[tb] = [xsb[:, c * 512:(c + 1) * 512] for c in range(NC)]
        # Split the prologue loads across both HWDGE queues (SP + ACT) for
        # ~2x DMA bandwidth before the ACT engine has real work, at c-chunk
        # granularity so the first Q-proj group unblocks ASAP.
        wq3 = wq_big[:].rearrange("p (c o) -> p c o", o=OL)
        wq3s = wqT.rearrange("(c p) o -> p c o", p=128)
        wk3 = wk_big[:].rearrange("p (c o) -> p c o", o=OL)
        wk3s = wkT.rearrange("(c p) o -> p c o", p=128)
        wv3 = wv_big[:].rearrange("p (c o) -> p c o", o=OL)
        wv3s = wvT.rearrange("(c p) o -> p c o", p=128)
        xsb0 = px.tile([128, NC * 512], bf16, tag="xsB", name="xsB_0")
        xs3 = xsb0[:].rearrange("p (c t) -> p c t", t=512)
        xs3s = xT[:, 0:512].rearrange("(c p) t -> p c t", p=128)
        for c in range(NC):
            nc.sync.dma_start(wq3[:, c:c + 1], wq3s[:, c:c + 1])
            nc.sync.dma_start(xs3[:, c:c + 1], xs3s[:, c:c + 1])
            nc.scalar.dma_start(wk3[:, c:c + 1], wk3s[:, c:c + 1])
            nc.scalar.dma_start(wv3[:, c:c + 1], wv3s[:, c:c + 1])
        xs_all[0] = [xsb0[:, c * 512:(c + 1) * 512] for c in range(NC)]
        nc.sync.dma_start(wo_big[:].rearrange("p (o c) -> p o c", c=C),
                          woT.rearrange("(o p) c -> p o c", p=128))
        nc.scalar.dma_start(tri_sb[:], triT[:, :])
        for t in range(JT):
            nc.gpsimd.memset(v_sb[t][:], 1.0)

        pmsk = ctx.enter_context(tc.tile_pool(name="pmsk", bufs=2))
        ppp = ctx.enter_context(tc.tile_pool(name="ppp", bufs=12))
        pnrm = ctx.enter_context(tc.tile_pool(name="pnrm", bufs=2))
        pys = ctx.enter_context(tc.tile_pool(name="pys", bufs=3))
        psA = ctx.enter_context(tc.tile_pool(name="psA", bufs=2, space="PSUM"))
        psS = ctx.enter_context(tc.tile_pool(name="psS", bufs=2, space="PSUM"))
        psO = ctx.enter_context(tc.tile_pool(name="psO", bufs=2, space="PSUM"))

        def emit_oproj(ib_src, ct, on_act=False):
            iis = slice(ib_src * 512, (ib_src + 1) * 512)
            psy = psA.tile([128, 512], f32, tag="pq", name=f"psy_{ib_src}_{ct}")
            for o in range(NO):
                nc.tensor.matmul(psy[:],
                                 wo_sb[o][:, ct * 128:(ct + 1) * 128],
                                 aT_sb[o][:, iis],
                                 start=(o == 0), stop=(o == NO - 1))
            ys = pys.tile([128, 512], bf16, tag="ys", name=f"ys_{ib_src}_{ct}")
            if on_act:
                nc.scalar.copy(ys[:], psy[:])
            else:
                nc.vector.tensor_copy(ys[:], psy[:])
            nc.gpsimd.dma_start(yT[ct * 128:(ct + 1) * 128, iis], ys[:])

        pending = []
        for tb in range(IBN):
            tis = slice(tb * 512, (tb + 1) * 512)
            # ---- projections for this 512-col t-block ----
            xs = xs_all[tb]
            # early t-blocks: evacuate Q/K proj PSUM on ACT (idle before the
            # exp stream ramps); late t-blocks: DVE (ACT is exp-saturated).
            pcopy = nc.scalar.copy if tb < 2 else nc.vector.tensor_copy
            for o in range(NO):
                psq = psA.tile([128, 512], f32, tag="pq", name=f"psq{o}_{tb}")
                for c in range(NC):
                    nc.tensor.matmul(psq[:], wq_sb[c][:, o * 128:(o + 1) * 128],
                                     xs[c][:], start=(c == 0), stop=(c == NC - 1))
                pcopy(qT_sb[o][:, tis], psq[:])
            for o in range(NO):
                psk = psA.tile([128, 512], f32, tag="pq", name=f"psk{o}_{tb}")
                for c in range(NC):
                    nc.tensor.matmul(psk[:], wk_sb[c][:, o * 128:(o + 1) * 128],
                                     xs[c][:], start=(c == 0), stop=(c == NC - 1))
                pcopy(kT_sb[o][:, tis], psk[:])
            for sub in range(4):
                t = tb * 4 + sub
                psv = psA.tile([128, OL], f32, tag="pq", name=f"psv_{t}")
                for c in range(NC):
                    nc.tensor.matmul(psv[:], xs[c][:, sub * 128:(sub + 1) * 128],
                                     wv_sb[c][:], start=(c == 0), stop=(c == NC - 1))
                v3 = v_sb[t][:].rearrange("p (h e) -> p h e", e=E)
                ps3 = psv[:].rearrange("p (h e) -> p h e", e=D)
                nc.vector.tensor_copy(v3[:, :, 0:D], ps3[:])
            if tb + 1 < IBN:
                load_xs(tb + 1)   # prefetch next block's x during attention

            # ---- attention for query block ib = tb ----
            ib = tb
            ii = slice(ib * 512, (ib + 1) * 512)
            js = [jt for jt in range(JT) if types[jt][ib] != 0]
            mx = {jt: mixidx[jt][ib] for jt in js if types[jt][ib] == 2}
            msk = {}
            for sl, (jt, m) in enumerate(mx.items()):
                mt = pmsk.tile([128, 512], bf16, tag=f"msk{sl}", name=f"msk{sl}_{ib}")
                nc.sync.dma_start(mt[:], maskT[m])
                msk[jt] = mt
            nj = len(js)
            for p in range(NP):
                # o_proj chunks are PE filler for the ACT-bound exp stream;
                # late query blocks have the most exp work and no projection
                # filler left, so weight the emission schedule toward them.
                # Emit them at LOW priority so score matmuls (which feed the
                # exp stream) always win the PE queue.
                with tc.high_priority(offset=-(1 << 20)):
                    for _ in range({0: 0, 1: 0, 2: 1, 3: 5}.get(ib, 2)):
                        if pending:
                            emit_oproj(*pending.pop(0))
                hA, hB = 2 * p, 2 * p + 1
                poA = psO.tile([65, 512], f32, tag="po", name=f"poA_{ib}_{p}")
                poB = psO.tile([65, 512], f32, tag="po", name=f"poB_{ib}_{p}")
                for cidx, jt in enumerate(js):
                    jj = slice(jt * 128, (jt + 1) * 128)
                    ts = (jt - 4 * ib) * 128 if types[jt][ib] == 3 else 0
                    qs = slice(ib * 512 + ts, (ib + 1) * 512)
                    sAB = psS.tile([128, 1024], f32, tag="sS",
                                   name=f"s_{ib}_{p}_{cidx}")
                    pAB = ppp.tile([128, 1024], bf16, tag="pP",
                                   name=f"pp_{ib}_{p}_{cidx}")
                    # scores + exp are the ACT-stream feed — highest priority
                    # so the PE always services them before filler work.
                    with tc.high_priority():
                        nc.tensor.matmul(sAB[:, ts:512], kT_sb[p][0:64, jj],
                                         qT_sb[p][0:64, qs],
                                         start=True, stop=True,
                                         tile_position=(0, 0))
                        nc.tensor.matmul(sAB[:, 512 + ts:1024],
                                         kT_sb[p][64:128, jj],
                                         qT_sb[p][64:128, qs],
                                         start=True, stop=True,
                                         tile_position=(64, 0))
                        if ts == 0:
                            nc.scalar.activation(pAB[:], sAB[:], EXP,
                                                 scale=SCALE)
                        else:
                            nc.scalar.activation(pAB[:, ts:512],
                                                 sAB[:, ts:512],
                                                 EXP, scale=SCALE)
                            nc.scalar.activation(pAB[:, 512 + ts:1024],
                                                 sAB[:, 512 + ts:1024],
                                                 EXP, scale=SCALE)
                    if types[jt][ib] == 3:
                        nc.vector.tensor_mul(pAB[:, ts:ts + 128],
                                             pAB[:, ts:ts + 128], tri_sb[:])
                        nc.vector.tensor_mul(pAB[:, 512 + ts:512 + ts + 128],
                                             pAB[:, 512 + ts:512 + ts + 128],
                                             tri_sb[:])
                    elif jt in msk:
                        nc.vector.tensor_mul(pAB[:, 0:512], pAB[:, 0:512],
                                             msk[jt][:])
                        nc.vector.tensor_mul(pAB[:, 512:1024], pAB[:, 512:1024],
                                             msk[jt][:])
                    first = (cidx == 0)
                    last = (cidx == nj - 1)
                    nc.tensor.matmul(poA[:, ts:512],
                                     v_sb[jt][:, hA * E:(hA + 1) * E],
                                     pAB[:, ts:512], start=first, stop=last)
                    nc.tensor.matmul(poB[:, ts:512],
                                     v_sb[jt][:, hB * E:(hB + 1) * E],
                                     pAB[:, 512 + ts:1024], start=first, stop=last)
                dnA = pnrm.tile([1, 512], f32, tag="dnA", name=f"dnA_{ib}_{p}")
                dnB = pnrm.tile([1, 512], f32, tag="dnB", name=f"dnB_{ib}_{p}")
                nc.vector.tensor_copy(dnA[:], poA[64:65, :])
                if ib == IBN - 1 and p == NP - 1:
                    # tail chain: stage the B denominator via ACT (idle by
                    # now) so the A/B normalization chains overlap.
                    nc.scalar.copy(dnB[:], poB[64:65, :])
                else:
                    nc.vector.tensor_copy(dnB[:], poB[64:65, :])
                rrA = pnrm.tile([1, 512], f32, tag="rrA", name=f"rrA_{ib}_{p}")
                rrB = pnrm.tile([1, 512], f32, tag="rrB", name=f"rrB_{ib}_{p}")
                nc.vector.reciprocal_approx_fast(rrA[:], dnA[:])
                nc.vector.reciprocal_approx_fast(rrB[:], dnB[:])
                bcA = pnrm.tile([64, 512], f32, tag="bcA", name=f"bcA_{ib}_{p}")
                bcB = pnrm.tile([64, 512], f32, tag="bcB", name=f"bcB_{ib}_{p}")
                nc.gpsimd.partition_broadcast(bcA[:], rrA[:])
                nc.gpsimd.partition_broadcast(bcB[:], rrB[:])
                nc.vector.tensor_mul(aT_sb[p][0:64, ii], poA[0:64, :], bcA[:])
                nc.vector.tensor_mul(aT_sb[p][64:128, ii], poB[0:64, :], bcB[:])
            pending.extend((ib, ct) for ct in range(NC))
        for i, (ib_src, ct) in enumerate(pending):
            emit_oproj(ib_src, ct, on_act=(i % 2 == 1))
        if debug:
            for o in range(NO):
                nc.sync.dma_start(dbg["qT"][o], qT_sb[o][:])
                nc.sync.dma_start(dbg["kT"][o], kT_sb[o][:])
                nc.sync.dma_start(dbg["aT"][o], aT_sb[o][:])
            for t in range(JT):
                nc.sync.dma_start(dbg["v"][t], v_sb[t][:])

    nc.compile()
    return nc


def classify_mask(mask2d, T):
    """mask2d: [T, T] (i=query rows, j=key cols).

    Returns types, mixidx, tiles where types[jt][ib] in {0 skip, 1 full,
    2 general-mixed, 3 causal-diagonal}."""
    JT, IBN = T // 128, T // 512
    types = [[0] * IBN for _ in range(JT)]
    mixidx = [[-1] * IBN for _ in range(JT)]
    tiles = []
    for jt in range(JT):
        for ib in range(IBN):
            blk = mask2d[ib * 512:(ib + 1) * 512, jt * 128:(jt + 1) * 128]
            if not blk.any():
                types[jt][ib] = 0
            elif blk.all():
                types[jt][ib] = 1
            else:
                dd = jt - 4 * ib
                if 0 <= dd < 4:
                    want = np.zeros((512, 128), dtype=blk.dtype)
                    want[dd * 128:(dd + 1) * 128, :] = np.tril(
                        np.ones((128, 128), dtype=blk.dtype))
                    if (dd + 1) * 128 < 512:
                        want[(dd + 1) * 128:, :] = 1
                    if np.array_equal(blk != 0, want != 0):
                        types[jt][ib] = 3
                        continue
                types[jt][ib] = 2
                mixidx[jt][ib] = len(tiles)
                tiles.append(np.ascontiguousarray(blk.T.astype(np.float32)))
    tiles = np.stack(tiles) if tiles else None
    return types, mixidx, tiles


B, T, C = 4, 2048, 1024
H, HD = 16, 64
G = 2
HL = H // G
OL = HL * HD

_cache = {}
_last_run = {}


def kernel(x, mask, wq, wk, wv, wo):
    import ml_dtypes
    from concourse import bass_utils
    bf = ml_dtypes.bfloat16
    x = np.asarray(x, dtype=np.float32)
    mask = np.asarray(mask)
    wq = np.asarray(wq, dtype=np.float32)
    wk = np.asarray(wk, dtype=np.float32)
    wv = np.asarray(wv, dtype=np.float32)
    wo = np.asarray(wo, dtype=np.float32)

    mask2d = mask.reshape(mask.shape[-2], mask.shape[-1])
    types, mixidx, tiles = classify_mask(mask2d, T)
    nmix = 0 if tiles is None else len(tiles)

    key = tuple(tuple(r) for r in types)
    if key not in _cache:
        _cache[key] = build(T=T, C=C, HL=HL, D=HD, types=types, mixidx=mixidx,
                            nmix=nmix, n_cores=8)
    nc = _cache[key]

    tri = np.triu(np.ones((128, 128), np.float32)).astype(bf)
    in_maps = []
    for b in range(B):
        for g in range(G):
            m = {
                "xT": np.ascontiguousarray(x[b].T).astype(bf),
                "wqT": np.ascontiguousarray(wq[g * OL:(g + 1) * OL, :].T).astype(bf),
                "wkT": np.ascontiguousarray(wk[g * OL:(g + 1) * OL, :].T).astype(bf),
                "wvT": np.ascontiguousarray(wv[g * OL:(g + 1) * OL, :].T).astype(bf),
                "woT": np.ascontiguousarray(wo[:, g * OL:(g + 1) * OL].T).astype(bf),
                "triT": tri,
            }
            if nmix:
                m["maskT"] = tiles.astype(bf)
            in_maps.append(m)

    _last_run["nc"] = nc
    _last_run["in_maps"] = in_maps
    res = bass_utils.run_bass_kernel_spmd(nc, in_maps, core_ids=list(range(8)))
    out = np.empty((B, T, C), np.float32)
    for b in range(B):
        out[b] = (res.results[2 * b]["yT"].astype(np.float32)
                  + res.results[2 * b + 1]["yT"].astype(np.float32)).T
    return out
